# revision 14
# baseline (speedup 1.0000x reference)
"""Trainium2 Bass kernel for nn_MultiHeadAttention_67044439491211.

Mathematical note: the reference einsum 'bqkh,bvha->bqha' sums k and v
independently, so attn = (sum_k softmax(...)) * (sum_v v) = sum_v v
(softmax sums to 1 over k).  The whole module therefore collapses to

    out[b, q, :] = (sum_c context[b, c, :]) @ Wkv[:, D:] @ Wout

independent of q, query, Wq and mask.  The device kernel computes the
context reduction and the (folded) weight matmul, then broadcasts the
row across the q dimension and writes the full output shard.

Sharding: core c handles batch b = c//2 and output rows
[(c%2)*1024, (c%2+1)*1024).  Each core reads the full context of its
batch (needed for the complete reduction), so context is read twice
across the 8 cores.

Pipeline (v2): the context partial reduction runs on the PE as an
accumulating ones-vector matmul chain (csum[1,512] in PSUM), consuming
each 512KB DMA unit as it lands -- the DVE add chain it replaces lagged
the DMA stream by ~4.5us.  The row is transposed to partition layout
with four k=1 matmuls, multiplied against bf16-folded weights (single
pass; tolerance is 2e-2), broadcast across PSUM rows by a
column-broadcast stationary operand, and written out as two
8KB-descriptor DMAs (one per HWDGE ring).
"""

import numpy as np
import ml_dtypes

from concourse import bacc
import concourse.mybir as mybir
from concourse.tile import TileContext
from concourse.bass_utils import run_bass_kernel_spmd

B, QL, CL, D, H = 4, 2048, 2048, 512, 8
N_CORES = 8
ROWS_PER_CORE = QL // 2  # 1024

F32 = mybir.dt.float32
F32R = mybir.dt.float32r
BF16 = mybir.dt.bfloat16

_NC_CACHE = {}


def _build_nc():
    nc = bacc.Bacc("TRN2", target_bir_lowering=False, enable_partition_id=False,
                   monotonic_sem_count=0)

    ctx_h = nc.dram_tensor("ctx", [CL, D], F32R, kind="ExternalInput")
    # host passes W2 = Wv @ Wout in SBUF layout: [p, c*512+n] = W2[c*128+p, n]
    w2_h = nc.dram_tensor("w2", [128, 4 * D], BF16, kind="ExternalInput")
    out_h = nc.dram_tensor("out", [ROWS_PER_CORE, D], F32, kind="ExternalOutput")

    P = 128
    G = 4            # context DMA units (1 MB each)
    NT = 4           # consecutive rows per partition (G*P*NT == CL); the
                     # per-partition contiguous run (= DMA descriptor) is NT*2KB
                     # (4KB descriptors measured ~215GB/s vs ~420 at 8KB)
    DC = D // P      # 4 column chunks of 128

    # DRAM views: partition p reads NT consecutive rows (NT*2KB contiguous)
    # per unit, one descriptor each.  First 3 units are 1MB (8KB desc, the
    # fast regime); the last two are 0.5MB (4KB desc) so the final
    # reduction matmuls start sooner after the stream ends.
    ctx_big = ctx_h[0:1536, :].rearrange("(g p n) d -> g p (n d)", g=3, p=P, n=4)
    ctx_sml = ctx_h[1536:2048, :].rearrange("(g p n) d -> g p (n d)", g=2, p=P, n=2)

    with TileContext(nc) as tc:
        with (
            tc.tile_pool(name="ctxp", bufs=G) as ctxp,
            tc.tile_pool(name="work", bufs=1) as work,
            tc.tile_pool(name="psum", bufs=1, space="PSUM") as psum,
        ):
            # context first on the sync HWDGE ring; weights queue behind
            # (needed ~2us after the last ctx byte, so they just make it)
            tiles = []  # (tile, n_chunks)
            for g in range(3):
                t = ctxp.tile([P, 4 * D], F32R, tag="ctx")
                nc.sync.dma_start(out=t[:], in_=ctx_big[g])
                tiles.append((t, 4))
            for g in range(2):
                t = ctxp.tile([P, 2 * D], F32R, tag="ctxs")
                nc.sync.dma_start(out=t[:], in_=ctx_sml[g])
                tiles.append((t, 2))
            w2_sb = work.tile([P, DC * D], BF16, tag="w2_sb")
            nc.sync.dma_start(out=w2_sb[:], in_=w2_h[:, :])

            # constants (memset can't write f32r; copy-cast from f32)
            ones1f = work.tile([P, 1], F32, tag="ones1f")
            nc.vector.memset(ones1f[:], 1.0)
            ones1 = work.tile([P, 1], F32R, tag="ones1")
            nc.vector.tensor_copy(out=ones1[:], in_=ones1f[:])
            onepf = work.tile([1, 1], F32, tag="onepf")
            nc.vector.memset(onepf[:], 1.0)
            onep = work.tile([1, 1], BF16, tag="onep")
            nc.vector.tensor_copy(out=onep[:], in_=onepf[:])

            # PE warm-up: HAM holds TensorE at 1.2 GHz until ~4us of
            # sustained work; run a throwaway accumulation chain on a junk
            # tile while the first ctx unit is in flight so the real
            # reduction runs at 2.4 GHz.
            junk = work.tile([P, D], F32, tag="junk")
            nc.vector.memset(junk[:], 0.0)
            scratch_ps = psum.tile([1, D], F32, tag="scratch_ps")
            for w in range(3):  # f32: 4-pass, ~1.7us each at 1.2GHz
                nc.tensor.matmul(scratch_ps[:], ones1f[:], junk[:],
                                 start=True, stop=True)

            # csum[0, d] = sum_rows ctx[row, d]: accumulating PE matmul
            # chain, ones [128,1] stationary, each 512-col chunk streamed
            # as it lands (~315ns/chunk warm; PE keeps pace with the DMA)
            csum_ps = psum.tile([1, D], F32, tag="csum_ps")
            n_mm = sum(n for _, n in tiles)
            i = 0
            for t, n_chunks in tiles:
                for k in range(n_chunks):
                    nc.tensor.matmul(
                        csum_ps[:],
                        ones1[:],
                        t[:, k * D : (k + 1) * D],
                        start=(i == 0),
                        stop=(i == n_mm - 1),
                    )
                    i += 1

            csum_sb = work.tile([1, D], BF16, tag="csum_sb")
            nc.vector.tensor_copy(out=csum_sb[:], in_=csum_ps[:])

            # transpose to partition layout: csumT[m, c] = csum[0, c*128+m]
            # via four k=1 rank-1 matmuls (lhsT = csum slice [1, 128]);
            # bf16 — the f32 version cost 2.6us in LDWEIGHTS/MATMUL slices
            csumT_ps = psum.tile([P, DC], F32, tag="csumT_ps")
            for c in range(DC):
                nc.tensor.matmul(
                    csumT_ps[:, c : c + 1],
                    csum_sb[:, c * P : (c + 1) * P],
                    onep[:],
                    start=True,
                    stop=True,
                )
            csT_bf = work.tile([P, DC], BF16, tag="csT_bf")
            nc.vector.tensor_copy(out=csT_bf[:], in_=csumT_ps[:])

            # o-matmuls with a column-broadcast stationary operand:
            # lhsT[k, m] = csumT[k, c] for every m, so every output row of
            # the (128, 512) PSUM tile is o[n] — the q-broadcast falls out
            # of the matmul for free.  Single bf16 pass (~1e-3 rel err).
            bc_ps = psum.tile([P, D], F32, tag="bc_ps")
            for c in range(DC):
                nc.tensor.matmul(
                    bc_ps[:],
                    csT_bf[:, c : c + 1].broadcast_to([P, P]),
                    w2_sb[:, c * D : (c + 1) * D],
                    start=(c == 0),
                    stop=(c == DC - 1),
                )

            bcast = work.tile([P, D], F32, tag="bcast")
            nc.vector.tensor_copy(out=bcast[:], in_=bc_ps[:])

            # three output DMAs: sync HWDGE first (first byte ~1us after
            # issue), then scalar (~3.1us) and gpsimd SWDGE (~3.9us),
            # sized so they finish together under the shared ~436GB/s
            # SDMA bus.  All 128 bcast rows are identical, so partition p
            # can own J consecutive DRAM rows: 6-8KB descriptors.
            a = bcast[:]
            segs = ((0, 384, 3, nc.sync), (384, 768, 3, nc.scalar),
                    (768, 1024, 2, nc.gpsimd))
            for r0, r1, J, ring in segs:
                out_v = out_h[r0:r1, :].rearrange("(p j) n -> p (j n)", p=P, j=J)
                rep = type(a)(a.tensor, a.offset, [a.ap[0], [0, J], a.ap[1]])
                ring.dma_start(out=out_v, in_=rep)

    nc.compile()
    return nc


def kernel(query=None, context=None, mask=None, Wq=None, Wkv=None, Wout=None,
           trace=False, **_ignored):
    context = np.asarray(context, dtype=np.float32)
    Wkv = np.asarray(Wkv, dtype=np.float32)
    Wout = np.asarray(Wout, dtype=np.float32)

    # fold the V projection and output projection into one matrix
    W2 = (Wkv[:, D:].astype(np.float64) @ Wout.astype(np.float64)).astype(np.float32)
    # pre-layout to SBUF shape: [p, c*512+n] = W2[c*128+p, n]
    W2sb = np.ascontiguousarray(
        W2.reshape(4, 128, D).transpose(1, 0, 2).reshape(128, 4 * D)
    )
    w2bf = W2sb.astype(ml_dtypes.bfloat16)

    if "nc" not in _NC_CACHE:
        _NC_CACHE["nc"] = _build_nc()
    nc = _NC_CACHE["nc"]

    in_maps = []
    for c in range(N_CORES):
        b = c // 2
        in_maps.append({"ctx": np.ascontiguousarray(context[b]), "w2": w2bf})

    res = run_bass_kernel_spmd(nc, in_maps, core_ids=list(range(N_CORES)),
                               trace=trace)
    kernel.last_results = res

    out = np.empty((B, QL, D), dtype=np.float32)
    for c in range(N_CORES):
        b, h = c // 2, c % 2
        out[b, h * ROWS_PER_CORE : (h + 1) * ROWS_PER_CORE, :] = res.results[c]["out"]
    return out


kernel.last_results = None


# revision 16
# speedup vs baseline: 1.0095x; 1.0095x over previous
"""Trainium2 Bass kernel for nn_MultiHeadAttention_67044439491211.

Mathematical note: the reference einsum 'bqkh,bvha->bqha' sums k and v
independently, so attn = (sum_k softmax(...)) * (sum_v v) = sum_v v
(softmax sums to 1 over k).  The whole module therefore collapses to

    out[b, q, :] = (sum_c context[b, c, :]) @ Wkv[:, D:] @ Wout

independent of q, query, Wq and mask.  The device kernel computes the
context reduction and the (folded) weight matmul, then broadcasts the
row across the q dimension and writes the full output shard.

Sharding: core c handles batch b = c//2 and output rows
[(c%2)*1024, (c%2+1)*1024).  Each core reads the full context of its
batch (needed for the complete reduction), so context is read twice
across the 8 cores.

Pipeline (v2): the context partial reduction runs on the PE as an
accumulating ones-vector matmul chain (csum[1,512] in PSUM), consuming
each 512KB DMA unit as it lands -- the DVE add chain it replaces lagged
the DMA stream by ~4.5us.  The row is transposed to partition layout
with four k=1 matmuls, multiplied against bf16-folded weights (single
pass; tolerance is 2e-2), broadcast across PSUM rows by a
column-broadcast stationary operand, and written out as two
8KB-descriptor DMAs (one per HWDGE ring).
"""

import numpy as np
import ml_dtypes

from concourse import bacc
import concourse.mybir as mybir
from concourse.tile import TileContext
from concourse.bass_utils import run_bass_kernel_spmd

B, QL, CL, D, H = 4, 2048, 2048, 512, 8
N_CORES = 8
ROWS_PER_CORE = QL // 2  # 1024

F32 = mybir.dt.float32
F32R = mybir.dt.float32r
BF16 = mybir.dt.bfloat16

_NC_CACHE = {}


def _build_nc():
    nc = bacc.Bacc("TRN2", target_bir_lowering=False, enable_partition_id=False,
                   monotonic_sem_count=0)

    ctx_h = nc.dram_tensor("ctx", [CL, D], F32R, kind="ExternalInput")
    # host passes W2 = Wv @ Wout in SBUF layout: [p, c*512+n] = W2[c*128+p, n]
    w2_h = nc.dram_tensor("w2", [128, 4 * D], BF16, kind="ExternalInput")
    out_h = nc.dram_tensor("out", [ROWS_PER_CORE, D], F32, kind="ExternalOutput")

    P = 128
    G = 4            # context DMA units (1 MB each)
    NT = 4           # consecutive rows per partition (G*P*NT == CL); the
                     # per-partition contiguous run (= DMA descriptor) is NT*2KB
                     # (4KB descriptors measured ~215GB/s vs ~420 at 8KB)
    DC = D // P      # 4 column chunks of 128

    # DRAM views: partition p reads NT consecutive rows (NT*2KB contiguous)
    # per unit, one descriptor each.  First 3 units are 1MB (8KB desc, the
    # fast regime); the last two are 0.5MB (4KB desc) so the final
    # reduction matmuls start sooner after the stream ends.
    ctx_big = ctx_h[0:1536, :].rearrange("(g p n) d -> g p (n d)", g=3, p=P, n=4)
    ctx_sml = ctx_h[1536:2048, :].rearrange("(g p n) d -> g p (n d)", g=2, p=P, n=2)

    with TileContext(nc) as tc:
        with (
            tc.tile_pool(name="ctxp", bufs=G) as ctxp,
            tc.tile_pool(name="work", bufs=1) as work,
            tc.tile_pool(name="psum", bufs=1, space="PSUM") as psum,
        ):
            # context first on the sync HWDGE ring; weights queue behind
            # (needed ~2us after the last ctx byte, so they just make it)
            tiles = []  # (tile, n_chunks)
            for g in range(3):
                t = ctxp.tile([P, 4 * D], F32R, tag="ctx")
                nc.sync.dma_start(out=t[:], in_=ctx_big[g])
                tiles.append((t, 4))
            for g in range(2):
                t = ctxp.tile([P, 2 * D], F32R, tag="ctxs")
                nc.sync.dma_start(out=t[:], in_=ctx_sml[g])
                tiles.append((t, 2))
            w2_sb = work.tile([P, DC * D], BF16, tag="w2_sb")
            nc.sync.dma_start(out=w2_sb[:], in_=w2_h[:, :])

            # constants (memset can't write f32r; copy-cast from f32)
            ones1f = work.tile([P, 1], F32, tag="ones1f")
            nc.vector.memset(ones1f[:], 1.0)
            ones1 = work.tile([P, 1], F32R, tag="ones1")
            nc.vector.tensor_copy(out=ones1[:], in_=ones1f[:])
            onepf = work.tile([1, 1], F32, tag="onepf")
            nc.vector.memset(onepf[:], 1.0)
            onep = work.tile([1, 1], BF16, tag="onep")
            nc.vector.tensor_copy(out=onep[:], in_=onepf[:])

            # PE warm-up + pstate hold: HAM keeps TensorE at 1.2 GHz until
            # ~4us of SUSTAINED work, and drops it back (with a ~2us stall)
            # after ~3us idle.  Run a throwaway f32 chain on a junk tile
            # while the first ctx unit is in flight, then sprinkle fp32r
            # filler matmuls wherever the PE would otherwise idle >1us.
            junk = work.tile([P, D], F32, tag="junk")
            nc.vector.memset(junk[:], 0.0)
            junk_r = work.tile([P, D], F32R, tag="junk_r")
            nc.vector.tensor_copy(out=junk_r[:], in_=junk[:])
            scratch_ps = psum.tile([1, D], F32, tag="scratch_ps")
            for w in range(3):  # f32: 4-pass, ~1.7us each at 1.2GHz
                nc.tensor.matmul(scratch_ps[:], ones1f[:], junk[:],
                                 start=True, stop=True)

            def filler(n):
                for _ in range(n):
                    nc.tensor.matmul(scratch_ps[:], ones1[:], junk_r[:],
                                     start=True, stop=True)

            # csum[0, d] = sum_rows ctx[row, d]: accumulating PE matmul
            # chain, ones [128,1] stationary, each 512-col chunk streamed
            # as it lands (~390ns/chunk at 2.4GHz; tracks the DMA)
            csum_ps = psum.tile([1, D], F32, tag="csum_ps")
            n_mm = sum(n for _, n in tiles)
            i = 0
            for t, n_chunks in tiles:
                for k in range(n_chunks):
                    nc.tensor.matmul(
                        csum_ps[:],
                        ones1[:],
                        t[:, k * D : (k + 1) * D],
                        start=(i == 0),
                        stop=(i == n_mm - 1),
                    )
                    i += 1
                if i < n_mm:
                    filler(4)

            csum_sb = work.tile([1, D], BF16, tag="csum_sb")
            nc.vector.tensor_copy(out=csum_sb[:], in_=csum_ps[:])
            filler(3)  # cover the DVE cast

            # transpose to partition layout: csumT[m, c] = csum[0, c*128+m]
            # via four k=1 rank-1 matmuls (lhsT = csum slice [1, 128]);
            # bf16 — the f32 version cost 2.6us in LDWEIGHTS/MATMUL slices
            csumT_ps = psum.tile([P, DC], F32, tag="csumT_ps")
            for c in range(DC):
                nc.tensor.matmul(
                    csumT_ps[:, c : c + 1],
                    csum_sb[:, c * P : (c + 1) * P],
                    onep[:],
                    start=True,
                    stop=True,
                )
            csT_bf = work.tile([P, DC], BF16, tag="csT_bf")
            nc.vector.tensor_copy(out=csT_bf[:], in_=csumT_ps[:])

            # o-matmuls with a column-broadcast stationary operand:
            # lhsT[k, m] = csumT[k, c] for every m, so every output row of
            # the (128, 512) PSUM tile is o[n] — the q-broadcast falls out
            # of the matmul for free.  Single bf16 pass (~1e-3 rel err).
            bc_ps = psum.tile([P, D], F32, tag="bc_ps")
            for c in range(DC):
                nc.tensor.matmul(
                    bc_ps[:],
                    csT_bf[:, c : c + 1].broadcast_to([P, P]),
                    w2_sb[:, c * D : (c + 1) * D],
                    start=(c == 0),
                    stop=(c == DC - 1),
                )

            bcast = work.tile([P, D], F32, tag="bcast")
            nc.vector.tensor_copy(out=bcast[:], in_=bc_ps[:])

            # three output DMAs: sync HWDGE first (first byte ~1us after
            # issue), then scalar (~3.1us) and gpsimd SWDGE (~3.9us),
            # sized so they finish together under the shared ~436GB/s
            # SDMA bus.  All 128 bcast rows are identical, so partition p
            # can own J consecutive DRAM rows: 6-8KB descriptors.
            a = bcast[:]
            segs = ((0, 384, 3, nc.sync), (384, 768, 3, nc.scalar),
                    (768, 1024, 2, nc.gpsimd))
            for r0, r1, J, ring in segs:
                out_v = out_h[r0:r1, :].rearrange("(p j) n -> p (j n)", p=P, j=J)
                rep = type(a)(a.tensor, a.offset, [a.ap[0], [0, J], a.ap[1]])
                ring.dma_start(out=out_v, in_=rep)

    nc.compile()
    return nc


def kernel(query=None, context=None, mask=None, Wq=None, Wkv=None, Wout=None,
           trace=False, **_ignored):
    context = np.asarray(context, dtype=np.float32)
    Wkv = np.asarray(Wkv, dtype=np.float32)
    Wout = np.asarray(Wout, dtype=np.float32)

    # fold the V projection and output projection into one matrix
    W2 = (Wkv[:, D:].astype(np.float64) @ Wout.astype(np.float64)).astype(np.float32)
    # pre-layout to SBUF shape: [p, c*512+n] = W2[c*128+p, n]
    W2sb = np.ascontiguousarray(
        W2.reshape(4, 128, D).transpose(1, 0, 2).reshape(128, 4 * D)
    )
    w2bf = W2sb.astype(ml_dtypes.bfloat16)

    if "nc" not in _NC_CACHE:
        _NC_CACHE["nc"] = _build_nc()
    nc = _NC_CACHE["nc"]

    in_maps = []
    for c in range(N_CORES):
        b = c // 2
        in_maps.append({"ctx": np.ascontiguousarray(context[b]), "w2": w2bf})

    res = run_bass_kernel_spmd(nc, in_maps, core_ids=list(range(N_CORES)),
                               trace=trace)
    kernel.last_results = res

    out = np.empty((B, QL, D), dtype=np.float32)
    for c in range(N_CORES):
        b, h = c // 2, c % 2
        out[b, h * ROWS_PER_CORE : (h + 1) * ROWS_PER_CORE, :] = res.results[c]["out"]
    return out


kernel.last_results = None


# revision 21
# speedup vs baseline: 1.0571x; 1.0471x over previous
"""Trainium2 Bass kernel for nn_MultiHeadAttention_67044439491211.

Mathematical note: the reference einsum 'bqkh,bvha->bqha' sums k and v
independently, so attn = (sum_k softmax(...)) * (sum_v v) = sum_v v
(softmax sums to 1 over k).  The whole module therefore collapses to

    out[b, q, :] = (sum_c context[b, c, :]) @ Wkv[:, D:] @ Wout

independent of q, query, Wq and mask.  The device kernel computes the
context reduction and the (folded) weight matmul, then broadcasts the
row across the q dimension and writes the full output shard.

Sharding: core c handles batch b = c//2 and output rows
[(c%2)*1024, (c%2+1)*1024).  Each core reads the full context of its
batch (needed for the complete reduction), so context is read twice
across the 8 cores.

Pipeline: the context reduction runs on the PE as an accumulating
ones-vector fp32r matmul chain (csum[1,512] in PSUM), consuming each
1MB DMA unit as it lands (the DVE add chain it replaced lagged the DMA
stream by ~4.5us).  The row is transposed to partition layout with
four k=1 bf16 matmuls, multiplied against bf16-folded weights (single
pass; tolerance is 2e-2), broadcast across PSUM rows by a
column-broadcast stationary operand, and written out as two
8KB-descriptor DMAs (sync HWDGE + gpsimd SWDGE; the scalar ring's
first byte lands ~3.1us after issue).
"""

import numpy as np
import ml_dtypes

from concourse import bacc
import concourse.mybir as mybir
from concourse.tile import TileContext
from concourse.bass_utils import run_bass_kernel_spmd

B, QL, CL, D, H = 4, 2048, 2048, 512, 8
N_CORES = 8
ROWS_PER_CORE = QL // 2  # 1024

F32 = mybir.dt.float32
F32R = mybir.dt.float32r
BF16 = mybir.dt.bfloat16

_NC_CACHE = {}


def _build_nc():
    nc = bacc.Bacc("TRN2", target_bir_lowering=False, enable_partition_id=False,
                   monotonic_sem_count=0)

    ctx_h = nc.dram_tensor("ctx", [CL, D], F32R, kind="ExternalInput")
    # host passes W2 = Wv @ Wout in SBUF layout: [p, c*512+n] = W2[c*128+p, n]
    w2_h = nc.dram_tensor("w2", [128, 4 * D], BF16, kind="ExternalInput")
    out_h = nc.dram_tensor("out", [ROWS_PER_CORE, D], F32, kind="ExternalOutput")

    P = 128
    G = 4            # context DMA units (1 MB each)
    NT = 4           # consecutive rows per partition -> 8KB descriptors
                     # (4KB descriptors measured ~215GB/s vs ~420 at 8KB)
    DC = D // P      # 4 column chunks of 128

    ctx_v = ctx_h[:, :].rearrange("(g p n) d -> g p (n d)", g=G, p=P, n=NT)

    with TileContext(nc) as tc:
        with (
            tc.tile_pool(name="ctxp", bufs=G) as ctxp,
            tc.tile_pool(name="work", bufs=1) as work,
            tc.tile_pool(name="psum", bufs=1, space="PSUM") as psum,
        ):
            # context first on the sync HWDGE ring; weights queue behind
            tiles = []
            for g in range(G):
                t = ctxp.tile([P, NT * D], F32R, tag="ctx")
                nc.sync.dma_start(out=t[:], in_=ctx_v[g])
                tiles.append(t)
            w2_sb = work.tile([P, DC * D], BF16, tag="w2_sb")
            nc.sync.dma_start(out=w2_sb[:], in_=w2_h[:, :])

            # constants (memset can't write f32r; copy-cast from f32)
            ones1f = work.tile([P, 1], F32, tag="ones1f")
            nc.vector.memset(ones1f[:], 1.0)
            ones1 = work.tile([P, 1], F32R, tag="ones1")
            nc.vector.tensor_copy(out=ones1[:], in_=ones1f[:])
            onepf = work.tile([1, 1], F32, tag="onepf")
            nc.vector.memset(onepf[:], 1.0)
            onep = work.tile([1, 1], BF16, tag="onep")
            nc.vector.tensor_copy(out=onep[:], in_=onepf[:])

            # csum[0, d] = sum_rows ctx[row, d]: accumulating PE matmul
            # chain, ones [128,1] stationary, each 512-col chunk streamed
            # as it lands
            csum_ps = psum.tile([1, D], F32, tag="csum_ps")
            n_mm = G * NT
            i = 0
            for t in tiles:
                for k in range(NT):
                    nc.tensor.matmul(
                        csum_ps[:],
                        ones1[:],
                        t[:, k * D : (k + 1) * D],
                        start=(i == 0),
                        stop=(i == n_mm - 1),
                    )
                    i += 1

            csum_sb = work.tile([1, D], BF16, tag="csum_sb")
            nc.vector.tensor_copy(out=csum_sb[:], in_=csum_ps[:])

            # transpose to partition layout: csumT[m, c] = csum[0, c*128+m]
            # via four k=1 rank-1 bf16 matmuls (lhsT = csum slice [1, 128])
            csumT_ps = psum.tile([P, DC], F32, tag="csumT_ps")
            for c in range(DC):
                nc.tensor.matmul(
                    csumT_ps[:, c : c + 1],
                    csum_sb[:, c * P : (c + 1) * P],
                    onep[:],
                    start=True,
                    stop=True,
                )
            csT_bf = work.tile([P, DC], BF16, tag="csT_bf")
            nc.vector.tensor_copy(out=csT_bf[:], in_=csumT_ps[:])

            # o-matmuls with a column-broadcast stationary operand:
            # lhsT[k, m] = csumT[k, c] for every m, so every output row of
            # the (128, 512) PSUM tile is o[n] — the q-broadcast falls out
            # of the matmul for free.  Single bf16 pass (~3e-3 rel err).
            bc_ps = psum.tile([P, D], F32, tag="bc_ps")
            for c in range(DC):
                nc.tensor.matmul(
                    bc_ps[:],
                    csT_bf[:, c : c + 1].broadcast_to([P, P]),
                    w2_sb[:, c * D : (c + 1) * D],
                    start=(c == 0),
                    stop=(c == DC - 1),
                )

            bcast = work.tile([P, D], F32, tag="bcast")
            nc.vector.tensor_copy(out=bcast[:], in_=bc_ps[:])

            # two output DMAs: sync HWDGE + gpsimd SWDGE.  All 128 bcast
            # rows are identical, so partition p can own 4 consecutive
            # DRAM rows: 8KB descriptors.
            half_rows = ROWS_PER_CORE // 2  # 512 rows per queue
            J = half_rows // P              # 4 rows per partition
            out_v = out_h[:, :].rearrange("(q p j) n -> q p (j n)", q=2, p=P, j=J)
            a = bcast[:]
            for q, ring in ((0, nc.sync), (1, nc.gpsimd)):
                rep = type(a)(a.tensor, a.offset, [a.ap[0], [0, J], a.ap[1]])
                ring.dma_start(out=out_v[q], in_=rep)

    nc.compile()
    return nc


def kernel(query=None, context=None, mask=None, Wq=None, Wkv=None, Wout=None,
           trace=False, **_ignored):
    context = np.asarray(context, dtype=np.float32)
    Wkv = np.asarray(Wkv, dtype=np.float32)
    Wout = np.asarray(Wout, dtype=np.float32)

    # fold the V projection and output projection into one matrix
    W2 = (Wkv[:, D:].astype(np.float64) @ Wout.astype(np.float64)).astype(np.float32)
    # pre-layout to SBUF shape: [p, c*512+n] = W2[c*128+p, n]
    W2sb = np.ascontiguousarray(
        W2.reshape(4, 128, D).transpose(1, 0, 2).reshape(128, 4 * D)
    )
    w2bf = W2sb.astype(ml_dtypes.bfloat16)

    if "nc" not in _NC_CACHE:
        _NC_CACHE["nc"] = _build_nc()
    nc = _NC_CACHE["nc"]

    in_maps = []
    for c in range(N_CORES):
        b = c // 2
        in_maps.append({"ctx": np.ascontiguousarray(context[b]), "w2": w2bf})

    res = run_bass_kernel_spmd(nc, in_maps, core_ids=list(range(N_CORES)),
                               trace=trace)
    kernel.last_results = res

    out = np.empty((B, QL, D), dtype=np.float32)
    for c in range(N_CORES):
        b, h = c // 2, c % 2
        out[b, h * ROWS_PER_CORE : (h + 1) * ROWS_PER_CORE, :] = res.results[c]["out"]
    return out


kernel.last_results = None
